# revision 12
# baseline (speedup 1.0000x reference)
"""Trainium2 Bass kernel for per-head-projection MultiHeadAttention.

Contract: kernel(**inputs) takes the FULL unsharded inputs (as produced by
reference.setup_inputs()) and returns the FULL [B, S, D] output.

Sharding (tensor-parallel over heads x data-parallel over batch):
  - 8 cores; cores 0-3 handle batch 0, cores 4-7 handle batch 1.
  - Each core owns 4 heads (two "head pairs"). It computes Q/K/V projections
    for those heads, causal attention, and a partial output projection
    (ctx @ Wo rows for its heads). The host sums the 4 partials per batch
    (the output linear is linear over head blocks) and adds bo.

Device layouts (chosen so no on-device transposes of activations are needed
except a cheap PE transpose of V):
  - inputs are fed pre-transposed: x^T [D, S] per batch.
  - projections produce Q^T/K^T/V^T [128(=2 heads of 64), S] directly.
  - scores are computed transposed ([keys, queries]); softmax denominator
    comes free from a ones-column appended to V; exp runs on ScalarE with
    the 1/sqrt(DH) scale fused in.
All matmuls run as float32r (full-rate fp32 on trn2 for free dim >= 256).
"""

import os
import sys

sys.path.insert(0, "/opt/trn_rl_repo")

import numpy as np

B, S, D, H = 2, 2048, 1024, 16
DH = D // H            # 64
NCORES = 8
HPC = H * B // NCORES  # 4 heads per core
NPAIR = HPC // 2       # 2 head pairs per core
SG = 512               # query-group size
NSG = S // SG          # 4
NKT = S // 128         # 16 key tiles
NDC = D // 128         # 8 contraction chunks

_BUILD_CACHE = {}


def _build(causal: bool):
    """Build + compile the per-core Bass program. Cached per causal flag."""
    import concourse.bacc as bacc
    import concourse.tile as tile
    from concourse import mybir

    f32 = mybir.dt.float32
    f32r = mybir.dt.float32r
    EXP = mybir.ActivationFunctionType.Exp

    nc = bacc.Bacc("TRN2", target_bir_lowering=False, debug=False)

    xq = nc.dram_tensor("xq", [D, S], f32r, kind="ExternalInput").ap()
    xk = nc.dram_tensor("xk", [D, S], f32r, kind="ExternalInput").ap()
    xv = nc.dram_tensor("xv", [D, S], f32r, kind="ExternalInput").ap()
    wq = nc.dram_tensor("wq", [NPAIR, D, 128], f32r, kind="ExternalInput").ap()
    wk = nc.dram_tensor("wk", [NPAIR, D, 128], f32r, kind="ExternalInput").ap()
    wv = nc.dram_tensor("wv", [NPAIR, D, 128], f32r, kind="ExternalInput").ap()
    wo = nc.dram_tensor("wo", [NPAIR, 128, D], f32r, kind="ExternalInput").ap()
    mk = nc.dram_tensor("mk", [128, 512], f32r, kind="ExternalInput").ap()
    on = nc.dram_tensor("on", [128, 64], f32r, kind="ExternalInput").ap()
    idm = nc.dram_tensor("idm", [128, 64], f32r, kind="ExternalInput").ap()
    bq = nc.dram_tensor("bq", [NPAIR, 128, 1], f32, kind="ExternalInput").ap()
    bk = nc.dram_tensor("bk", [NPAIR, 128, 1], f32, kind="ExternalInput").ap()
    bv = nc.dram_tensor("bv", [NPAIR, 128, 1], f32, kind="ExternalInput").ap()
    out = nc.dram_tensor("out", [S, D], f32, kind="ExternalOutput").ap()

    with tile.TileContext(nc) as tc:
        with (
            tc.tile_pool(name="persist", bufs=1) as persist,
            tc.tile_pool(name="xs", bufs=4) as xs_pool,
            tc.tile_pool(name="pts", bufs=4) as pt_pool,
            tc.tile_pool(name="outs", bufs=3) as out_pool,
            tc.tile_pool(name="smalls", bufs=3) as small_pool,
            tc.tile_pool(name="psma", bufs=4, space="PSUM") as psA,
            tc.tile_pool(name="psmb", bufs=2, space="PSUM") as psB,
        ):
            # 64x64 identity blocks at base partitions 0 and 64 (matmul
            # requires lhsT/rhs to share a base partition). Host-provided.
            ident = persist.tile([128, 64], f32r, tag="ident")
            nc.sync.dma_start(out=ident, in_=idm)

            # Diagonal-tile multiplier, host-provided: cols 0-383 are 0
            # (fully masked blocks), cols 384-511 are tri(j <= i) for the
            # diagonal 128-block. Slicing [.., (3-tp)*128 : 512] yields the
            # right multiplier for diagonal offset tp.
            mask = persist.tile([128, 512], f32r, tag="mask")
            nc.sync.dma_start(out=mask, in_=mk)

            w_sb = persist.tile([128, 3, NPAIR, NDC, 128], f32r, tag="w")
            for t_i, wd in enumerate([wq, wk, wv]):
                for p in range(NPAIR):
                    for c in range(NDC):
                        nc.sync.dma_start(
                            out=w_sb[:, t_i, p, c, :],
                            in_=wd[p, c * 128 : (c + 1) * 128, :],
                        )
            wo_sb = persist.tile([128, NPAIR, D], f32r, tag="wo")
            for p in range(NPAIR):
                nc.sync.dma_start(out=wo_sb[:, p, :], in_=wo[p])
            b_sb = persist.tile([128, 3, NPAIR], f32, tag="b")
            for t_i, bd in enumerate([bq, bk, bv]):
                for p in range(NPAIR):
                    nc.sync.dma_start(out=b_sb[:, t_i, p : p + 1], in_=bd[p])

            qT = persist.tile([128, NPAIR, S], f32r, tag="qT")
            kT = persist.tile([128, NPAIR, S], f32r, tag="kT")
            vT = persist.tile([128, NPAIR, S], f32r, tag="vT")
            vN = persist.tile([128, HPC, NKT, 65], f32r, tag="vN")
            ctxn = persist.tile([128, NPAIR, S], f32r, tag="ctxn")

            # ---- Phase A: projections -> Q^T / K^T / V^T [128(pair), S] ----
            for t_i, (xd, dest) in enumerate([(xq, qT), (xk, kT), (xv, vT)]):
                for g in range(NSG):
                    ps = [
                        psA.tile([128, SG], f32, tag="mm", name="ps_proj")
                        for _ in range(NPAIR)
                    ]
                    for c in range(NDC):
                        xc = xs_pool.tile([128, SG], f32r, tag="xc")
                        nc.sync.dma_start(
                            out=xc,
                            in_=xd[c * 128 : (c + 1) * 128, g * SG : (g + 1) * SG],
                        )
                        xcr = xc
                        for p in range(NPAIR):
                            nc.tensor.matmul(
                                ps[p],
                                lhsT=w_sb[:, t_i, p, c, :],
                                rhs=xcr,
                                start=(c == 0),
                                stop=(c == NDC - 1),
                            )
                    for p in range(NPAIR):
                        nc.vector.tensor_scalar_add(
                            out=dest[:, p, g * SG : (g + 1) * SG],
                            in0=ps[p],
                            scalar1=b_sb[:, t_i, p : p + 1],
                        )

            # ---- V to natural layout [keys, 65] (ones column -> denom) ----
            nc.sync.dma_start(
                out=vN[:, :, :, 64], in_=on.rearrange("p (h k) -> p h k", h=HPC)
            )
            for p in range(NPAIR):
                for h_s in range(2):
                    h = p * 2 + h_s
                    hp = slice(h_s * 64, (h_s + 1) * 64)
                    for kt in range(NKT):
                        tp = psB.tile([128, 64], f32r, tag="tp")
                        nc.tensor.transpose(
                            tp,
                            in_=vT[hp, p, kt * 128 : (kt + 1) * 128],
                            identity=ident[hp, :],
                        )
                        nc.vector.tensor_copy(out=vN[:, h, kt, 0:64], in_=tp)

            # ---- Phase B: attention per head ----
            for p in range(NPAIR):
                for h_s in range(2):
                    h = p * 2 + h_s
                    hp = slice(h_s * 64, (h_s + 1) * 64)
                    for g in range(NSG):
                        nkc = (4 * g + 4) if causal else NKT
                        ctx = psB.tile([65, SG], f32, tag="ctx")
                        # Software-pipelined: emit pv(kc-1) after sc/exp(kc)
                        # so the in-order PE always has the next scores
                        # matmul while ACT runs exp.
                        pts = [None] * nkc
                        for kc in range(nkc):
                            sc = psA.tile([128, SG], f32, tag="mm", name="sc")
                            nc.tensor.matmul(
                                sc,
                                lhsT=kT[hp, p, kc * 128 : (kc + 1) * 128],
                                rhs=qT[hp, p, g * SG : (g + 1) * SG],
                                start=True,
                                stop=True,
                            )
                            pt = pt_pool.tile([128, SG], f32r, tag="pt", name="pt")
                            tp_i = kc - 4 * g
                            nc.scalar.activation(pt, sc, EXP, scale=0.125)
                            if causal and tp_i >= 0:
                                ncol = (tp_i + 1) * 128
                                nc.vector.tensor_mul(
                                    pt[:, 0:ncol],
                                    pt[:, 0:ncol],
                                    mask[:, SG - ncol : SG],
                                )
                            pts[kc] = pt
                            if kc >= 1:
                                nc.tensor.matmul(
                                    ctx,
                                    lhsT=vN[:, h, kc - 1, :],
                                    rhs=pts[kc - 1],
                                    start=(kc - 1 == 0),
                                    stop=False,
                                )
                        nc.tensor.matmul(
                            ctx,
                            lhsT=vN[:, h, nkc - 1, :],
                            rhs=pts[nkc - 1],
                            start=(nkc - 1 == 0),
                            stop=True,
                        )
                        recip = small_pool.tile([1, SG], f32, tag="recip")
                        nc.vector.reciprocal(recip, ctx[64:65, :])
                        rb = small_pool.tile([64, SG], f32, tag="rb")
                        nc.gpsimd.partition_broadcast(rb, recip)
                        nc.vector.tensor_mul(
                            ctxn[hp, p, g * SG : (g + 1) * SG], ctx[0:64, :], rb
                        )

            # ---- Phase C: partial output projection ----
            for st in range(NKT):
                for n in range(D // SG):
                    op = psA.tile([128, SG], f32, tag="mm")
                    for p in range(NPAIR):
                        nc.tensor.matmul(
                            op,
                            lhsT=ctxn[:, p, st * 128 : (st + 1) * 128],
                            rhs=wo_sb[:, p, n * SG : (n + 1) * SG],
                            start=(p == 0),
                            stop=(p == NPAIR - 1),
                        )
                    ob = out_pool.tile([128, SG], f32, tag="ob")
                    nc.vector.tensor_copy(ob, op)
                    nc.sync.dma_start(
                        out=out[st * 128 : (st + 1) * 128, n * SG : (n + 1) * SG],
                        in_=ob,
                    )

    nc.compile()
    return nc


def _core_inputs(query, key, value, Wq, bq, Wk, bk, Wv, bv, Wo, core):
    b = core // (NCORES // B)
    h0 = (core % (NCORES // B)) * HPC
    f32 = np.float32

    def packw(W):
        # [H, D, DH] -> per-pair [D, 128] stacks
        return np.ascontiguousarray(
            np.stack(
                [
                    np.concatenate([W[h0 + 2 * p], W[h0 + 2 * p + 1]], axis=1)
                    for p in range(NPAIR)
                ]
            ),
            dtype=f32,
        )

    def packb(bias):
        return np.ascontiguousarray(
            np.stack(
                [
                    np.concatenate([bias[h0 + 2 * p], bias[h0 + 2 * p + 1]])
                    for p in range(NPAIR)
                ]
            ).reshape(NPAIR, 128, 1),
            dtype=f32,
        )

    wo_p = np.ascontiguousarray(
        np.stack(
            [Wo[(h0 + 2 * p) * DH : (h0 + 2 * p + 2) * DH] for p in range(NPAIR)]
        ),
        dtype=f32,
    )
    jj, ii = np.meshgrid(np.arange(128), np.arange(128), indexing="ij")
    mk = np.zeros((128, 512), f32)
    mk[:, 384:512] = (jj <= ii).astype(f32)
    return {
        "mk": mk,
        "on": np.ones((128, 64), f32),
        "idm": np.concatenate([np.eye(64, dtype=f32)] * 2, axis=0),
        "xq": np.ascontiguousarray(query[b].T, dtype=f32),
        "xk": np.ascontiguousarray(key[b].T, dtype=f32),
        "xv": np.ascontiguousarray(value[b].T, dtype=f32),
        "wq": packw(Wq),
        "wk": packw(Wk),
        "wv": packw(Wv),
        "wo": wo_p,
        "bq": packb(bq),
        "bk": packb(bk),
        "bv": packb(bv),
    }


LAST_RESULTS = None


def kernel(query, key, value, Wq, bq, Wk, bk, Wv, bv, Wo, bo, look_ahead_mask):
    global LAST_RESULTS
    from concourse.bass_utils import run_bass_kernel_spmd

    query = np.asarray(query, dtype=np.float32)
    key = np.asarray(key, dtype=np.float32)
    value = np.asarray(value, dtype=np.float32)
    Wq, Wk, Wv = (np.asarray(a, dtype=np.float32) for a in (Wq, Wk, Wv))
    bq, bk, bv = (np.asarray(a, dtype=np.float32) for a in (bq, bk, bv))
    Wo = np.asarray(Wo, dtype=np.float32)
    bo = np.asarray(bo, dtype=np.float32)
    causal = bool(np.asarray(look_ahead_mask).item())

    if causal not in _BUILD_CACHE:
        _BUILD_CACHE[causal] = _build(causal)
    nc = _BUILD_CACHE[causal]

    in_maps = [
        _core_inputs(query, key, value, Wq, bq, Wk, bk, Wv, bv, Wo, c)
        for c in range(NCORES)
    ]
    res = run_bass_kernel_spmd(nc, in_maps, core_ids=list(range(NCORES)))
    LAST_RESULTS = res

    gpb = NCORES // B
    out = np.stack(
        [
            np.sum([res.results[b * gpb + i]["out"] for i in range(gpb)], axis=0)
            for b in range(B)
        ]
    )
    return (out + bo[None, None, :]).astype(np.float32)


# revision 25
# speedup vs baseline: 1.4206x; 1.4206x over previous
"""Trainium2 Bass kernel for per-head-projection MultiHeadAttention.

Contract: kernel(**inputs) takes the FULL unsharded inputs (as produced by
reference.setup_inputs()) and returns the FULL [B, S, D] output.

Sharding (tensor-parallel over heads x data-parallel over batch):
  - 8 cores; cores 0-3 handle batch 0, cores 4-7 handle batch 1.
  - Each core owns 4 heads (two "head pairs"). It computes Q/K/V projections
    for those heads, causal attention, and a partial output projection
    (ctx @ Wo rows for its heads). The host sums the 4 partials per batch
    (the output linear is linear over head blocks) and adds bo.

Device structure (v2, pipelined over 512-row s-groups g=0..3):
  proj(g) -> V-transpose(g) -> attention(q-group g, both head pairs
  interleaved) -> normalize(g) -> output-projection(g). Attention consumes
  only K/V rows <= (g+1)*512 (causal), so everything streams.

Layouts: inputs pre-transposed on host (x^T [D, S]); projections emit
Q^T/K^T/V^T [128(=head pair), S]; scores are computed transposed
([keys, queries]) so softmax needs no transposes; the denominator comes
from a ones-column in V; exp runs fused over both heads ([128,1024] PSUM)
with the 1/sqrt(DH) scale folded in. All matmuls are float32r (full-rate
fp32). Diagonal causal tiles trim both the QK and PV matmul widths.
"""

import os
import sys

sys.path.insert(0, "/opt/trn_rl_repo")

import numpy as np

B, S, D, H = 2, 2048, 1024, 16
DH = D // H            # 64
NCORES = 8
HPC = H * B // NCORES  # 4 heads per core
NPAIR = HPC // 2       # 2 head pairs per core
SG = 512               # s-group / query-group size
NSG = S // SG          # 4
NKT = S // 128         # 16 key tiles
NDC = D // 128         # 8 contraction chunks

_BUILD_CACHE = {}


def _build(causal: bool):
    """Build + compile the per-core Bass program. Cached per causal flag."""
    import concourse.bass as bass
    import concourse.bacc as bacc
    import concourse.tile as tile
    from concourse import mybir

    f32 = mybir.dt.float32
    f32r = mybir.dt.float32r
    EXP = mybir.ActivationFunctionType.Exp

    nc = bacc.Bacc("TRN2", target_bir_lowering=False, debug=False)

    xq = nc.dram_tensor("xq", [D, S], f32r, kind="ExternalInput").ap()
    xk = nc.dram_tensor("xk", [D, S], f32r, kind="ExternalInput").ap()
    xv = nc.dram_tensor("xv", [D, S], f32r, kind="ExternalInput").ap()
    wq = nc.dram_tensor("wq", [NPAIR, D, 128], f32r, kind="ExternalInput").ap()
    wk = nc.dram_tensor("wk", [NPAIR, D, 128], f32r, kind="ExternalInput").ap()
    wv = nc.dram_tensor("wv", [NPAIR, D, 128], f32r, kind="ExternalInput").ap()
    wo = nc.dram_tensor("wo", [NPAIR, 128, D], f32r, kind="ExternalInput").ap()
    mk = nc.dram_tensor("mk", [128, 512], f32r, kind="ExternalInput").ap()
    on = nc.dram_tensor("on", [128, 64], f32r, kind="ExternalInput").ap()
    idm = nc.dram_tensor("idm", [128, 64], f32r, kind="ExternalInput").ap()
    bq = nc.dram_tensor("bq", [NPAIR, 128, 1], f32, kind="ExternalInput").ap()
    bk = nc.dram_tensor("bk", [NPAIR, 128, 1], f32, kind="ExternalInput").ap()
    bv = nc.dram_tensor("bv", [NPAIR, 128, 1], f32, kind="ExternalInput").ap()
    out = nc.dram_tensor("out", [S, D], f32, kind="ExternalOutput").ap()
    # DRAM bounce for the denominator inverses: DMA from DRAM supports
    # partition-broadcast (step-0) APs; SBUF sources and gpsimd
    # partition_broadcast (which always reads partition 0 on HW) do not.
    dscr = nc.dram_tensor("dscr", [NSG, HPC, SG], f32).ap()

    with tile.TileContext(nc) as tc:
        with (
            tc.tile_pool(name="persist", bufs=1) as persist,
            tc.tile_pool(name="xs", bufs=8) as xs_pool,
            tc.tile_pool(name="pts", bufs=4) as pt_pool,
            tc.tile_pool(name="vts", bufs=2) as vt_pool,
            tc.tile_pool(name="cxu", bufs=2) as cxu_pool,
            tc.tile_pool(name="outs", bufs=3) as out_pool,
            tc.tile_pool(name="smalls", bufs=2) as st_pool,
            tc.tile_pool(name="psma", bufs=2, space="PSUM") as psA,
            tc.tile_pool(name="psmb", bufs=2, space="PSUM") as psB,
            tc.tile_pool(name="psmc", bufs=2, space="PSUM") as psC,
        ):
            # consts (host-provided: identity blocks, diag mask)
            ident = persist.tile([128, 64], f32r, tag="ident")
            nc.sync.dma_start(out=ident, in_=idm)
            mask = persist.tile([128, 512], f32r, tag="mask")
            nc.sync.dma_start(out=mask, in_=mk)

            w_sb = persist.tile([128, 3, NPAIR, NDC, 128], f32r, tag="w")
            for t_i, wd in enumerate([wq, wk, wv]):
                for p in range(NPAIR):
                    for c in range(NDC):
                        nc.sync.dma_start(
                            out=w_sb[:, t_i, p, c, :],
                            in_=wd[p, c * 128 : (c + 1) * 128, :],
                        )
            wo_sb = persist.tile([128, NPAIR, D], f32r, tag="wo")
            for p in range(NPAIR):
                nc.sync.dma_start(out=wo_sb[:, p, :], in_=wo[p])
            b_sb = persist.tile([128, 3, NPAIR], f32, tag="b")
            for t_i, bd in enumerate([bq, bk, bv]):
                for p in range(NPAIR):
                    nc.sync.dma_start(out=b_sb[:, t_i, p : p + 1], in_=bd[p])

            qT = persist.tile([128, NPAIR, S], f32r, tag="qT")
            kT = persist.tile([128, NPAIR, S], f32r, tag="kT")
            vN = persist.tile([128, HPC, NKT, 65], f32r, tag="vN")
            ctxn = persist.tile([128, NPAIR, S], f32r, tag="ctxn")

            # ones column of V-natural (softmax denominator trick)
            nc.sync.dma_start(
                out=vN[:, :, :, 64], in_=on.rearrange("p (h k) -> p h k", h=HPC)
            )

            for g in range(NSG):
                gs = slice(g * SG, (g + 1) * SG)

                # ---- projections for s-group g (both pairs share a 2-bank
                # PSUM tile: pair p in columns [p*SG:(p+1)*SG]) ----
                vtg = vt_pool.tile([128, NPAIR, SG], f32r, tag="vtg", name="vtg")
                for t_i, xd in enumerate([xq, xk, xv]):
                    pp = psA.tile([128, 2 * SG], f32, tag="sc2", name="pp")
                    for c in range(NDC):
                        xc = xs_pool.tile([128, SG], f32r, tag="xc", name="xc")
                        nc.sync.dma_start(
                            out=xc, in_=xd[c * 128 : (c + 1) * 128, gs]
                        )
                        for p in range(NPAIR):
                            nc.tensor.matmul(
                                pp[:, p * SG : (p + 1) * SG],
                                lhsT=w_sb[:, t_i, p, c, :],
                                rhs=xc,
                                start=(c == 0),
                                stop=(c == NDC - 1),
                            )
                    for p in range(NPAIR):
                        dst = (
                            qT[:, p, gs]
                            if t_i == 0
                            else (kT[:, p, gs] if t_i == 1 else vtg[:, p, :])
                        )
                        nc.vector.tensor_scalar_add(
                            out=dst,
                            in0=pp[:, p * SG : (p + 1) * SG],
                            scalar1=b_sb[:, t_i, p : p + 1],
                        )

                # ---- V -> natural layout for this group's 4 key tiles ----
                for p in range(NPAIR):
                    for h_s in range(2):
                        h = 2 * p + h_s
                        hp = slice(h_s * 64, (h_s + 1) * 64)
                        for k4 in range(4):
                            tp_ps = psC.tile([128, 64], f32r, tag="mm", name="tp")
                            nc.tensor.transpose(
                                tp_ps,
                                in_=vtg[hp, p, k4 * 128 : (k4 + 1) * 128],
                                identity=ident[hp, :],
                            )
                            nc.vector.tensor_copy(
                                out=vN[:, h, 4 * g + k4, 0:64], in_=tp_ps
                            )

                # ---- attention for q-group g ----
                stage = st_pool.tile([128, SG], f32, tag="stage", name="stage")
                nc.vector.memset(stage, 1.0)
                ctxu = cxu_pool.tile([128, NPAIR, SG], f32, tag="ctxu", name="ctxu")
                nkc = (4 * g + 4) if causal else NKT
                for p in range(NPAIR):
                    ctx2 = [
                        psB.tile([65, SG], f32, tag="ctx", name="ctx")
                        for _ in range(2)
                    ]

                    def emit_pv(kc, tp_i, pt2, ctx2=ctx2, p=p, nkc=nkc):
                        pvoff = tp_i * 128 if (causal and tp_i > 0) else 0
                        for h_s in range(2):
                            nc.tensor.matmul(
                                ctx2[h_s][:, pvoff:SG],
                                lhsT=vN[:, 2 * p + h_s, kc, :],
                                rhs=pt2[:, h_s * SG + pvoff : (h_s + 1) * SG],
                                start=(kc == 0),
                                stop=(kc == nkc - 1),
                            )

                    # software-pipelined: sc/exp(kc) emitted before pv(kc-1)
                    prev = None
                    for kc in range(nkc):
                        tp_i = kc - 4 * g
                        diag = causal and tp_i >= 0
                        off = min(tp_i * 128, 256) if diag else 0
                        sc2 = psA.tile([128, 2 * SG], f32, tag="sc2", name="sc2")
                        for h_s in range(2):
                            hp = slice(h_s * 64, (h_s + 1) * 64)
                            nc.tensor.matmul(
                                sc2[:, h_s * SG + off : (h_s + 1) * SG],
                                lhsT=kT[hp, p, kc * 128 : (kc + 1) * 128],
                                rhs=qT[hp, p, g * SG + off : (g + 1) * SG],
                                start=True,
                                stop=True,
                            )
                        pt2 = pt_pool.tile([128, 2 * SG], f32r, tag="pt", name="pt2")
                        if off == 0:
                            nc.scalar.activation(pt2, sc2, EXP, scale=0.125)
                        else:
                            for h_s in range(2):
                                nc.scalar.activation(
                                    pt2[:, h_s * SG + off : (h_s + 1) * SG],
                                    sc2[:, h_s * SG + off : (h_s + 1) * SG],
                                    EXP,
                                    scale=0.125,
                                )
                        if diag:
                            d0 = tp_i * 128
                            for h_s in range(2):
                                nc.vector.tensor_mul(
                                    pt2[:, h_s * SG + d0 : h_s * SG + d0 + 128],
                                    pt2[:, h_s * SG + d0 : h_s * SG + d0 + 128],
                                    mask[:, 384:512],
                                )
                        if prev is not None:
                            emit_pv(*prev)
                        prev = (kc, tp_i, pt2)
                    emit_pv(*prev)

                    # stash denominators (32-aligned rows of stage) + raw ctx
                    for h_s in range(2):
                        h = 2 * p + h_s
                        hp = slice(h_s * 64, (h_s + 1) * 64)
                        nc.vector.tensor_copy(
                            stage[32 * h : 32 * h + 1, :], ctx2[h_s][64:65, :]
                        )
                        nc.vector.tensor_copy(ctxu[hp, p, :], ctx2[h_s][0:64, :])

                # ---- normalize: one full-tile reciprocal (rows 32h hold the
                # denominators; the rest is memset filler), broadcast each
                # inverse row across partitions, one mul per head ----
                inv = st_pool.tile([128, SG], f32, tag="inv", name="inv")
                nc.vector.reciprocal(inv, stage)
                for h in range(HPC):
                    nc.sync.dma_start(
                        out=dscr[g, h], in_=inv[32 * h : 32 * h + 1, :]
                    )
                for p in range(NPAIR):
                    rb = st_pool.tile([128, SG], f32, tag="rb", name="rb")
                    for h_s in range(2):
                        nc.sync.dma_start(
                            out=rb[h_s * 64 : (h_s + 1) * 64, :],
                            in_=dscr[g, 2 * p + h_s].partition_broadcast(64),
                        )
                    nc.vector.tensor_mul(ctxn[:, p, gs], ctxu[:, p, :], rb)

                # ---- partial output projection for s-group g ----
                for st4 in range(4):
                    srow = (4 * g + st4) * 128
                    for n in range(D // SG):
                        op = psC.tile([128, SG], f32, tag="mm", name="op")
                        for p in range(NPAIR):
                            nc.tensor.matmul(
                                op,
                                lhsT=ctxn[:, p, srow : srow + 128],
                                rhs=wo_sb[:, p, n * SG : (n + 1) * SG],
                                start=(p == 0),
                                stop=(p == NPAIR - 1),
                            )
                        ob = out_pool.tile([128, SG], f32, tag="ob", name="ob")
                        nc.vector.tensor_copy(ob, op)
                        nc.sync.dma_start(
                            out=out[srow : srow + 128, n * SG : (n + 1) * SG],
                            in_=ob,
                        )

    nc.compile()
    return nc


def _core_inputs(query, key, value, Wq, bq, Wk, bk, Wv, bv, Wo, core):
    b = core // (NCORES // B)
    h0 = (core % (NCORES // B)) * HPC
    f32 = np.float32

    def packw(W):
        # [H, D, DH] -> per-pair [D, 128] stacks
        return np.ascontiguousarray(
            np.stack(
                [
                    np.concatenate([W[h0 + 2 * p], W[h0 + 2 * p + 1]], axis=1)
                    for p in range(NPAIR)
                ]
            ),
            dtype=f32,
        )

    def packb(bias):
        return np.ascontiguousarray(
            np.stack(
                [
                    np.concatenate([bias[h0 + 2 * p], bias[h0 + 2 * p + 1]])
                    for p in range(NPAIR)
                ]
            ).reshape(NPAIR, 128, 1),
            dtype=f32,
        )

    wo_p = np.ascontiguousarray(
        np.stack(
            [Wo[(h0 + 2 * p) * DH : (h0 + 2 * p + 2) * DH] for p in range(NPAIR)]
        ),
        dtype=f32,
    )
    jj, ii = np.meshgrid(np.arange(128), np.arange(128), indexing="ij")
    mkk = np.zeros((128, 512), f32)
    mkk[:, 384:512] = (jj <= ii).astype(f32)
    return {
        "mk": mkk,
        "on": np.ones((128, 64), f32),
        "idm": np.concatenate([np.eye(64, dtype=f32)] * 2, axis=0),
        "xq": np.ascontiguousarray(query[b].T, dtype=f32),
        "xk": np.ascontiguousarray(key[b].T, dtype=f32),
        "xv": np.ascontiguousarray(value[b].T, dtype=f32),
        "wq": packw(Wq),
        "wk": packw(Wk),
        "wv": packw(Wv),
        "wo": wo_p,
        "bq": packb(bq),
        "bk": packb(bk),
        "bv": packb(bv),
    }


LAST_RESULTS = None


def kernel(query, key, value, Wq, bq, Wk, bk, Wv, bv, Wo, bo, look_ahead_mask):
    global LAST_RESULTS
    from concourse.bass_utils import run_bass_kernel_spmd

    query = np.asarray(query, dtype=np.float32)
    key = np.asarray(key, dtype=np.float32)
    value = np.asarray(value, dtype=np.float32)
    Wq, Wk, Wv = (np.asarray(a, dtype=np.float32) for a in (Wq, Wk, Wv))
    bq, bk, bv = (np.asarray(a, dtype=np.float32) for a in (bq, bk, bv))
    Wo = np.asarray(Wo, dtype=np.float32)
    bo = np.asarray(bo, dtype=np.float32)
    causal = bool(np.asarray(look_ahead_mask).item())

    if causal not in _BUILD_CACHE:
        _BUILD_CACHE[causal] = _build(causal)
    nc = _BUILD_CACHE[causal]

    in_maps = [
        _core_inputs(query, key, value, Wq, bq, Wk, bk, Wv, bv, Wo, c)
        for c in range(NCORES)
    ]
    res = run_bass_kernel_spmd(nc, in_maps, core_ids=list(range(NCORES)))
    LAST_RESULTS = res

    gpb = NCORES // B
    out = np.stack(
        [
            np.sum([res.results[b * gpb + i]["out"] for i in range(gpb)], axis=0)
            for b in range(B)
        ]
    )
    return (out + bo[None, None, :]).astype(np.float32)
